# revision 4
# baseline (speedup 1.0000x reference)
"""AffineCoupling (dense MLP) Trainium2 kernel.

Reference computation (B=16384, D=1024, HALF=512, HID=4096):
    a = z[:, 0::2]; b = z[:, 1::2]
    s = relu(a @ W1s + b1s) @ W2s + b2s
    t = relu(a @ W1t + b1t) @ W2t + b2t
    b_out = b * exp(s) + t
    logdet = s.sum(axis=1)
    z_out = interleave(a, b_out)

Strategy: data-parallel batch shard across 8 cores (2048 rows each), all
params replicated. On device everything is kept feature-major ("transposed",
[feature, batch]) so MLP biases are per-partition scalars and no on-device
transposes are needed; the host passes a^T / b^T slices and re-interleaves
the outputs. Matmuls run in bf16 (fp32 PSUM accumulation) at the full PE
rate; L1 and L2 of each MLP are software-pipelined per 128-row HID chunk so
the tensor engine never idles. logdet (a cross-partition sum) is a
ones-vector matmul overlapped with the t branch.

DMA choreography (the startup is weight-delivery-bound): the s-branch
weights stream on the sync HWDGE ring in exact consumption order (a^T
tile, W1s in column quarters interleaved with W2s chunks) so the PE can
start ~17us in and never stalls on weights; the t-branch weights arrive
in parallel via the GpSimd SWDGE queue (issuing them from the scalar
engine would block ACT on queue-full and starve the relus).
"""

import numpy as np
import ml_dtypes

import concourse.bass as bass
import concourse.tile as tile
from concourse import bacc, mybir
from concourse import bass_utils
from concourse.bass import ts

AF = mybir.ActivationFunctionType
ALU = mybir.AluOpType

B, D, HID = 16384, 1024, 4096
HALF = D // 2            # 512
NCORES = 8
BC = B // NCORES         # 2048 batch rows per core
P = 128
TB = 512                 # batch tile (matmul moving free dim)
NB = BC // TB            # 4 batch tiles per core
KF = HALF // P           # 4 feature chunks (L1 contraction)
MH = HID // P            # 32 hidden chunks
MO = HALF // P           # 4 output chunks
QW = 1024                # w1s streamed in column quarters
NQ = HID // QW           # 4
MPQ = QW // P            # 8 m-chunks per quarter

F32 = mybir.dt.float32
BF16 = mybir.dt.bfloat16
NPBF = ml_dtypes.bfloat16

_CACHE: dict = {}


def _build_nc():
    from contextlib import ExitStack

    nc = bacc.Bacc("TRN2", target_bir_lowering=False, debug=False,
                   enable_asserts=False)

    at_d = nc.dram_tensor("at", [HALF, BC], BF16, kind="ExternalInput").ap()
    bt_d = nc.dram_tensor("bt", [HALF, BC], F32, kind="ExternalInput").ap()
    w1_d = {br: nc.dram_tensor(f"w1{br}", [HALF, HID], BF16,
                               kind="ExternalInput").ap() for br in "st"}
    w2_d = {br: nc.dram_tensor(f"w2{br}", [HID, HALF], BF16,
                               kind="ExternalInput").ap() for br in "st"}
    b1_d = {br: nc.dram_tensor(f"b1{br}", [P, MH], F32,
                               kind="ExternalInput").ap() for br in "st"}
    b2_d = {br: nc.dram_tensor(f"b2{br}", [P, MO], F32,
                               kind="ExternalInput").ap() for br in "st"}
    bout_d = nc.dram_tensor("boutt", [HALF, BC], F32, kind="ExternalOutput").ap()
    ld_d = nc.dram_tensor("logdet", [1, BC], F32, kind="ExternalOutput").ap()

    with tile.TileContext(nc) as tc, ExitStack() as ctx:
        wpool = ctx.enter_context(tc.tile_pool(name="w", bufs=1))
        cpool = ctx.enter_context(tc.tile_pool(name="c", bufs=1))
        apool = ctx.enter_context(tc.tile_pool(name="a", bufs=9))
        bpool = ctx.enter_context(tc.tile_pool(name="b", bufs=9))
        hpool = ctx.enter_context(tc.tile_pool(name="h", bufs=4))
        spool = ctx.enter_context(tc.tile_pool(name="s", bufs=6))
        epool = ctx.enter_context(tc.tile_pool(name="e", bufs=6))
        opool = ctx.enter_context(tc.tile_pool(name="o", bufs=4))
        psh = ctx.enter_context(tc.tile_pool(name="psh", bufs=3, space="PSUM"))
        pss = ctx.enter_context(tc.tile_pool(name="pss", bufs=1, space="PSUM"))
        psl = ctx.enter_context(tc.tile_pool(name="psl", bufs=1, space="PSUM"))

        def load_a(n):
            ncol = slice(n * TB, (n + 1) * TB)
            at_t = []
            for k in range(KF):
                t = apool.tile([P, TB], BF16, tag="at", name="at")
                nc.sync.dma_start(t[:], at_d[k * P:(k + 1) * P, ncol])
                at_t.append(t)
            return at_t

        def load_b(n):
            ncol = slice(n * TB, (n + 1) * TB)
            bt_t = []
            for k in range(KF):
                t = bpool.tile([P, TB], F32, tag="bt", name="bt")
                nc.sync.dma_start(t[:], bt_d[k * P:(k + 1) * P, ncol])
                bt_t.append(t)
            return bt_t

        w1q = {}   # w1q[br, k, q] : [P, QW]
        w2 = {}    # w2[br, k2]   : [P, HALF]
        b1 = {}
        b2 = {}

        # t-branch weights via SWDGE (GpSimd is idle all kernel; its queue
        # blocking on depth is harmless, and it runs in parallel with the
        # sync HWDGE ring below).
        for k in range(KF):
            for q in range(NQ):
                t = wpool.tile([P, QW], BF16, tag=f"w1t{k}q{q}",
                               name=f"w1t{k}q{q}")
                nc.gpsimd.dma_start(
                    t[:], w1_d["t"][k * P:(k + 1) * P, q * QW:(q + 1) * QW])
                w1q["t", k, q] = t
        t = cpool.tile([P, MH], F32, tag="b1t", name="b1t")
        nc.gpsimd.dma_start(t[:], b1_d["t"][:, :])
        b1["t"] = t
        for k2 in range(MH):
            t = wpool.tile([P, HALF], BF16, tag=f"w2t{k2}", name=f"w2t{k2}")
            nc.gpsimd.dma_start(t[:], w2_d["t"][k2 * P:(k2 + 1) * P, :])
            w2["t", k2] = t
        t = cpool.tile([P, MO], F32, tag="b2t", name="b2t")
        nc.gpsimd.dma_start(t[:], b2_d["t"][:, :])
        b2["t"] = t

        # sync HWDGE ring, in exact first-use order for the s branch of
        # batch tile 0.
        at_next = load_a(0)

        def w1s_quarter(k, q):
            t = wpool.tile([P, QW], BF16, tag=f"w1s{k}q{q}", name=f"w1s{k}q{q}")
            nc.sync.dma_start(
                t[:], w1_d["s"][k * P:(k + 1) * P, q * QW:(q + 1) * QW])
            w1q["s", k, q] = t

        def w2s_chunk(k2):
            t = wpool.tile([P, HALF], BF16, tag=f"w2s{k2}", name=f"w2s{k2}")
            nc.sync.dma_start(t[:], w2_d["s"][k2 * P:(k2 + 1) * P, :])
            w2["s", k2] = t

        for k in range(KF):
            w1s_quarter(k, 0)
        t = cpool.tile([P, MH], F32, tag="b1s", name="b1s")
        nc.sync.dma_start(t[:], b1_d["s"][:, :])
        b1["s"] = t
        ones = cpool.tile([P, 1], BF16, tag="ones", name="ones")
        nc.vector.memset(ones[:], 1.0)
        for k2 in range(0, 4):
            w2s_chunk(k2)
        for k in range(KF):
            w1s_quarter(k, 1)
        for k2 in range(4, 12):
            w2s_chunk(k2)
        for k in range(KF):
            w1s_quarter(k, 2)
        for k2 in range(12, 20):
            w2s_chunk(k2)
        for k in range(KF):
            w1s_quarter(k, 3)
        for k2 in range(20, MH):
            w2s_chunk(k2)
        t = cpool.tile([P, MO], F32, tag="b2s", name="b2s")
        nc.sync.dma_start(t[:], b2_d["s"][:, :])
        b2["s"] = t
        bt_next = load_b(0)

        for n in range(NB):
            ncol = slice(n * TB, (n + 1) * TB)
            at_t, bt_t = at_next, bt_next
            if n + 1 < NB:
                at_next = load_a(n + 1)
                bt_next = load_b(n + 1)

            s_tiles = []
            exp_tiles = []
            for br in "st":
                psS = [pss.tile([P, TB], F32, tag=f"pss{mo}", name=f"pss{mo}")
                       for mo in range(MO)]
                # L1 (h = relu(a@W1+b1), chunk m) feeding L2 (accumulate
                # h-chunk into S), software-pipelined: L2 for chunk m-1 is
                # issued after L1 for chunk m so the PE never waits on ACT.
                pend = None
                for m in range(MH):
                    ph = psh.tile([P, TB], F32, tag="psh", name="psh")
                    q, mq = m // MPQ, m % MPQ
                    for k in range(KF):
                        nc.tensor.matmul(ph[:], w1q[br, k, q][:, ts(mq, P)],
                                         at_t[k][:],
                                         start=(k == 0), stop=(k == KF - 1))
                    ht = hpool.tile([P, TB], BF16, tag="ht", name="ht")
                    nc.scalar.activation(ht[:], ph[:], AF.Relu,
                                         bias=b1[br][:, m:m + 1], scale=1.0)
                    if pend is not None:
                        pm, pht = pend
                        for mo in range(MO):
                            nc.tensor.matmul(psS[mo][:],
                                             w2[br, pm][:, ts(mo, P)], pht[:],
                                             start=(pm == 0), stop=False)
                    pend = (m, ht)
                    # logdet for this batch tile, overlapped into the t
                    # branch (s_tiles are ready by then).
                    if br == "t" and m == 4:
                        pld = psl.tile([1, TB], F32, tag="pld", name="pld")
                        for mo in range(MO):
                            nc.tensor.matmul(pld[:], ones[:],
                                             s_tiles[mo][:],
                                             start=(mo == 0),
                                             stop=(mo == MO - 1))
                        ldt = opool.tile([1, TB], F32, tag="ld", name="ld")
                        nc.vector.tensor_copy(ldt[:], pld[:])
                        nc.sync.dma_start(ld_d[0:1, ncol], ldt[:])
                pm, pht = pend
                for mo in range(MO):
                    nc.tensor.matmul(psS[mo][:], w2[br, pm][:, ts(mo, P)],
                                     pht[:], start=False, stop=True)

                if br == "s":
                    for mo in range(MO):
                        sb = spool.tile([P, TB], BF16, tag="sbf", name="sbf")
                        nc.vector.tensor_scalar_add(sb[:], psS[mo][:],
                                                    b2["s"][:, mo:mo + 1])
                        ex = epool.tile([P, TB], F32, tag="exps", name="exps")
                        nc.scalar.activation(ex[:], psS[mo][:], AF.Exp,
                                             bias=b2["s"][:, mo:mo + 1],
                                             scale=1.0)
                        s_tiles.append(sb)
                        exp_tiles.append(ex)
                else:
                    for mo in range(MO):
                        tmp = opool.tile([P, TB], F32, tag="tmp", name="tmp")
                        nc.vector.tensor_mul(tmp[:], bt_t[mo][:],
                                             exp_tiles[mo][:])
                        bo = opool.tile([P, TB], F32, tag="bout", name="bout")
                        nc.vector.scalar_tensor_tensor(
                            bo[:], psS[mo][:], b2["t"][:, mo:mo + 1], tmp[:],
                            op0=ALU.add, op1=ALU.add)
                        nc.sync.dma_start(bout_d[mo * P:(mo + 1) * P, ncol],
                                          bo[:])

    nc.compile()
    return nc


def _get_nc():
    if "nc" not in _CACHE:
        _CACHE["nc"] = _build_nc()
    return _CACHE["nc"]


def _prep_in_maps(z, W1s, b1s, W2s, b2s, W1t, b1t, W2t, b2t):
    z = np.asarray(z, dtype=np.float32)
    a = z[:, 0::2]
    AT = np.ascontiguousarray(a.T).astype(NPBF)       # [HALF, B]
    BT = np.ascontiguousarray(z[:, 1::2].T)           # [HALF, B] f32

    def prep_w(w):
        return np.ascontiguousarray(np.asarray(w, dtype=np.float32)).astype(NPBF)

    def prep_b1(b):
        return np.ascontiguousarray(np.asarray(b, np.float32).reshape(MH, P).T)

    def prep_b2(b):
        return np.ascontiguousarray(np.asarray(b, np.float32).reshape(MO, P).T)

    shared = {
        "w1s": prep_w(W1s), "w2s": prep_w(W2s),
        "w1t": prep_w(W1t), "w2t": prep_w(W2t),
        "b1s": prep_b1(b1s), "b2s": prep_b2(b2s),
        "b1t": prep_b1(b1t), "b2t": prep_b2(b2t),
    }
    in_maps = []
    for c in range(NCORES):
        sl = slice(c * BC, (c + 1) * BC)
        in_maps.append({
            "at": np.ascontiguousarray(AT[:, sl]),
            "bt": np.ascontiguousarray(BT[:, sl]),
            **shared,
        })
    return z, a, in_maps


def kernel(z, W1s, b1s, W2s, b2s, W1t, b1t, W2t, b2t):
    z, a, in_maps = _prep_in_maps(z, W1s, b1s, W2s, b2s, W1t, b1t, W2t, b2t)
    nc = _get_nc()
    res = bass_utils.run_bass_kernel_spmd(nc, in_maps,
                                          core_ids=list(range(NCORES)))

    boutt = np.concatenate([r["boutt"] for r in res.results], axis=1)  # [HALF, B]
    logdet = np.concatenate([r["logdet"][0] for r in res.results])     # [B]

    z_out = np.empty_like(z)
    z_out[:, 0::2] = a
    z_out[:, 1::2] = boutt.T
    return z_out, logdet


# revision 14
# speedup vs baseline: 1.1811x; 1.1811x over previous
"""AffineCoupling (dense MLP) Trainium2 kernel.

Reference computation (B=16384, D=1024, HALF=512, HID=4096):
    a = z[:, 0::2]; b = z[:, 1::2]
    s = relu(a @ W1s + b1s) @ W2s + b2s
    t = relu(a @ W1t + b1t) @ W2t + b2t
    b_out = b * exp(s) + t
    logdet = s.sum(axis=1)
    z_out = interleave(a, b_out)

Strategy: data-parallel batch shard across 8 cores (2048 rows each), all
params replicated. On device everything is kept feature-major ("transposed",
[feature, batch]) so MLP biases are per-partition scalars and no on-device
transposes are needed; the host passes a^T / b^T slices and re-interleaves
the outputs. Matmuls run in bf16 (fp32 PSUM accumulation) at the full PE
rate; L1 and L2 of each MLP are software-pipelined per 128-row HID chunk so
the tensor engine never idles. logdet (a cross-partition sum) is a
ones-vector matmul overlapped with the t branch.

DMA choreography (the startup is weight-delivery-bound): the s-branch
weights stream on the sync HWDGE ring in exact consumption order (a^T
tile, W1s in column quarters interleaved with W2s chunks) so the PE can
start ~17us in and never stalls on weights; the t-branch weights arrive
in parallel via the GpSimd SWDGE queue (issuing them from the scalar
engine would block ACT on queue-full and starve the relus).
"""

import numpy as np
import ml_dtypes

import concourse.bass as bass
import concourse.tile as tile
from concourse import bacc, mybir
from concourse import bass_utils
from concourse.bass import ts
from concourse.tile_rust import add_dep_helper

AF = mybir.ActivationFunctionType
ALU = mybir.AluOpType

B, D, HID = 16384, 1024, 4096
HALF = D // 2            # 512
NCORES = 8
BC = B // NCORES         # 2048 batch rows per core
P = 128
TB = 512                 # batch tile (matmul moving free dim)
NB = BC // TB            # 4 batch tiles per core
KF = HALF // P           # 4 feature chunks (L1 contraction)
MH = HID // P            # 32 hidden chunks
MO = HALF // P           # 4 output chunks
QW = 1024                # w1s streamed in column quarters
NQ = HID // QW           # 4
MPQ = QW // P            # 8 m-chunks per quarter

F32 = mybir.dt.float32
BF16 = mybir.dt.bfloat16
NPBF = ml_dtypes.bfloat16

_CACHE: dict = {}


def _build_nc():
    from contextlib import ExitStack

    nc = bacc.Bacc("TRN2", target_bir_lowering=False, debug=False,
                   enable_asserts=False)

    at_d = nc.dram_tensor("at", [HALF, BC], BF16, kind="ExternalInput").ap()
    bt_d = nc.dram_tensor("bt", [HALF, BC], F32, kind="ExternalInput").ap()
    w1_d = {br: nc.dram_tensor(f"w1{br}", [HALF, HID], BF16,
                               kind="ExternalInput").ap() for br in "st"}
    w2_d = {br: nc.dram_tensor(f"w2{br}", [HID, HALF], BF16,
                               kind="ExternalInput").ap() for br in "st"}
    b1_d = {br: nc.dram_tensor(f"b1{br}", [P, MH], F32,
                               kind="ExternalInput").ap() for br in "st"}
    b2_d = {br: nc.dram_tensor(f"b2{br}", [P, MO], F32,
                               kind="ExternalInput").ap() for br in "st"}
    bout_d = nc.dram_tensor("boutt", [HALF, BC], F32, kind="ExternalOutput").ap()
    ld_d = nc.dram_tensor("logdet", [1, BC], F32, kind="ExternalOutput").ap()

    with tile.TileContext(nc) as tc, ExitStack() as ctx:
        wpool = ctx.enter_context(tc.tile_pool(name="w", bufs=1))
        cpool = ctx.enter_context(tc.tile_pool(name="c", bufs=1))
        apool = ctx.enter_context(tc.tile_pool(name="a", bufs=9))
        bpool = ctx.enter_context(tc.tile_pool(name="b", bufs=9))
        hpool = ctx.enter_context(tc.tile_pool(name="h", bufs=4))
        spool = ctx.enter_context(tc.tile_pool(name="s", bufs=6))
        epool = ctx.enter_context(tc.tile_pool(name="e", bufs=6))
        opool = ctx.enter_context(tc.tile_pool(name="o", bufs=4))
        psh = ctx.enter_context(tc.tile_pool(name="psh", bufs=3, space="PSUM"))
        pss = ctx.enter_context(tc.tile_pool(name="pss", bufs=1, space="PSUM"))
        psl = ctx.enter_context(tc.tile_pool(name="psl", bufs=1, space="PSUM"))

        def load_a(n):
            ncol = slice(n * TB, (n + 1) * TB)
            at_t = []
            for k in range(KF):
                t = apool.tile([P, TB], BF16, tag="at", name="at")
                nc.sync.dma_start(t[:], at_d[k * P:(k + 1) * P, ncol])
                at_t.append(t)
            return at_t

        def load_b(n):
            ncol = slice(n * TB, (n + 1) * TB)
            bt_t = []
            for k in range(KF):
                t = bpool.tile([P, TB], F32, tag="bt", name="bt")
                nc.sync.dma_start(t[:], bt_d[k * P:(k + 1) * P, ncol])
                bt_t.append(t)
            return bt_t

        w1q = {}   # w1q[br, k, q] : [P, QW]
        w2 = {}    # w2[br, k2]   : [P, HALF]
        b1 = {}
        b2 = {}

        # PE warm-up: the HAM clock gate keeps the PE at 1.2GHz until it
        # has seen ~3.4us of sustained activity, and the first real matmul
        # can't start until weights arrive (~12us in). These dependency-free
        # dummy matmuls (zeros @ ones -> scratch PSUM) start immediately,
        # span the DMA window, and hand the real stream a 2.4GHz PE.
        ones = cpool.tile([P, 1], BF16, tag="ones", name="ones")
        nc.vector.memset(ones[:], 1.0)
        warm_sb = cpool.tile([P, 256], BF16, tag="warm_sb", name="warm_sb")
        nc.vector.memset(warm_sb[:], 0.0)
        warm_ps = psl.tile([P, 256], F32, tag="pld", name="warm_ps")
        for _ in range(44):
            nc.tensor.matmul(warm_ps[:], warm_sb[:, 0:P], warm_sb[:],
                             start=True, stop=True)
        # Pre-load the ACT spline tables (relu/exp) so the first real relu
        # doesn't pay the ~2.7us table load on the critical path.
        warm_act = cpool.tile([P, 1], F32, tag="warm_act", name="warm_act")
        nc.scalar.activation(warm_act[:], warm_sb[:, 0:1], AF.Relu)
        nc.scalar.activation(warm_act[:], warm_sb[:, 0:1], AF.Exp)

        # sync HWDGE ring, in exact first-use order for the s branch of
        # batch tile 0.
        at_next = load_a(0)

        w1s_head = {}

        def w1s_quarter(k, q):
            t = wpool.tile([P, QW], BF16, tag=f"w1s{k}q{q}", name=f"w1s{k}q{q}")
            nc.sync.dma_start(
                t[:], w1_d["s"][k * P:(k + 1) * P, q * QW:(q + 1) * QW])
            w1q["s", k, q] = t

        def w1s_q0_head(k):
            t = wpool.tile([P, 2 * P], BF16, tag=f"w1s{k}h", name=f"w1s{k}h")
            nc.sync.dma_start(t[:], w1_d["s"][k * P:(k + 1) * P, 0:2 * P])
            w1s_head[k] = t

        def w1s_q0_tail(k):
            t = wpool.tile([P, QW - 2 * P], BF16, tag=f"w1s{k}q0",
                           name=f"w1s{k}q0")
            ins = nc.sync.dma_start(
                t[:], w1_d["s"][k * P:(k + 1) * P, 2 * P:QW])
            w1q["s", k, 0] = t
            return ins

        def w2s_chunk(k2):
            t = wpool.tile([P, HALF], BF16, tag=f"w2s{k2}", name=f"w2s{k2}")
            ins = nc.sync.dma_start(t[:], w2_d["s"][k2 * P:(k2 + 1) * P, :])
            w2["s", k2] = t
            return ins

        for k in range(KF):
            w1s_q0_head(k)
        t = cpool.tile([P, MH], F32, tag="b1s", name="b1s")
        nc.sync.dma_start(t[:], b1_d["s"][:, :])
        b1["s"] = t
        tail_insts = [w1s_q0_tail(k) for k in range(KF)]
        w2s_insts = {}
        for k2 in range(0, 6):
            w2s_insts[k2] = w2s_chunk(k2)
        for k in range(KF):
            w1s_quarter(k, 1)
        for k2 in range(6, 10):
            w2s_chunk(k2)
        for k in range(KF):
            w1s_quarter(k, 2)
        for k2 in range(10, 20):
            w2s_insts[k2] = w2s_chunk(k2)
        for k in range(KF):
            w1s_quarter(k, 3)
        for k2 in range(20, MH):
            w2s_chunk(k2)
        t = cpool.tile([P, MO], F32, tag="b2s", name="b2s")
        nc.sync.dma_start(t[:], b2_d["s"][:, :])
        b2["s"] = t
        bt_next = load_b(0)

        # t-branch weights via SWDGE (GpSimd is idle all kernel). Gated
        # behind the startup-critical sync transfers: with no gate the two
        # queues split HBM bandwidth packet-for-packet and the PE waits
        # ~15us longer for its first weights; the t weights aren't needed
        # until ~95us so delaying them to ~15us costs nothing.
        swdge_first = None
        for k in range(KF):
            for q in range(NQ):
                t = wpool.tile([P, QW], BF16, tag=f"w1t{k}q{q}",
                               name=f"w1t{k}q{q}")
                ins = nc.gpsimd.dma_start(
                    t[:], w1_d["t"][k * P:(k + 1) * P, q * QW:(q + 1) * QW])
                if swdge_first is None:
                    swdge_first = ins
                w1q["t", k, q] = t
        t = cpool.tile([P, MH], F32, tag="b1t", name="b1t")
        nc.gpsimd.dma_start(t[:], b1_d["t"][:, :])
        b1["t"] = t
        for k2 in range(MH):
            t = wpool.tile([P, HALF], BF16, tag=f"w2t{k2}", name=f"w2t{k2}")
            nc.gpsimd.dma_start(t[:], w2_d["t"][k2 * P:(k2 + 1) * P, :])
            w2["t", k2] = t
        t = cpool.tile([P, MO], F32, tag="b2t", name="b2t")
        nc.gpsimd.dma_start(t[:], b2_d["t"][:, :])
        b2["t"] = t
        add_dep_helper(swdge_first.ins, w2s_insts[19].ins,
                       reason="hold SWDGE weight stream off the HBM until "
                              "the startup-critical sync transfers land")

        for n in range(NB):
            ncol = slice(n * TB, (n + 1) * TB)
            at_t, bt_t = at_next, bt_next
            if n + 1 < NB:
                at_next = load_a(n + 1)
                bt_next = load_b(n + 1)

            s_tiles = []
            exp_tiles = []
            for br in "st":
                psS = [pss.tile([P, TB], F32, tag=f"pss{mo}", name=f"pss{mo}")
                       for mo in range(MO)]
                # L1 (h = relu(a@W1+b1), chunk m) feeding L2 (accumulate
                # h-chunk into S), software-pipelined: L2 for chunk m-1 is
                # issued after L1 for chunk m so the PE never waits on ACT.
                pend = []
                for m in range(MH):
                    ph = psh.tile([P, TB], F32, tag="psh", name="psh")
                    q, mq = m // MPQ, m % MPQ
                    for k in range(KF):
                        if br == "s" and m < 2:
                            wsl = w1s_head[k][:, ts(m, P)]
                        elif br == "s" and q == 0:
                            wsl = w1q[br, k, 0][:, ts(m - 2, P)]
                        else:
                            wsl = w1q[br, k, q][:, ts(mq, P)]
                        nc.tensor.matmul(ph[:], wsl, at_t[k][:],
                                         start=(k == 0), stop=(k == KF - 1))
                    ht = hpool.tile([P, TB], BF16, tag="ht", name="ht")
                    nc.scalar.activation(ht[:], ph[:], AF.Relu,
                                         bias=b1[br][:, m:m + 1], scale=1.0)
                    if len(pend) == 2:
                        pm, pht = pend.pop(0)
                        for mo in range(MO):
                            nc.tensor.matmul(psS[mo][:],
                                             w2[br, pm][:, ts(mo, P)], pht[:],
                                             start=(pm == 0), stop=False)
                    pend.append((m, ht))
                    # logdet for this batch tile, overlapped into the t
                    # branch (s_tiles are ready by then).
                    if br == "t" and m == 4:
                        pld = psl.tile([1, TB], F32, tag="pld", name="pld")
                        for j, st_ in enumerate(s_tiles):
                            nc.tensor.matmul(pld[:], ones[:], st_[:],
                                             start=(j == 0),
                                             stop=(j == len(s_tiles) - 1))
                        ldt = opool.tile([1, TB], F32, tag="ld", name="ld")
                        nc.vector.tensor_copy(ldt[:], pld[:])
                        nc.sync.dma_start(ld_d[0:1, ncol], ldt[:])
                for pm, pht in pend:
                    for mo in range(MO):
                        nc.tensor.matmul(psS[mo][:], w2[br, pm][:, ts(mo, P)],
                                         pht[:], start=False,
                                         stop=(pm == MH - 1))

                if br == "s":
                    for mo in range(MO):
                        sb = spool.tile([P, TB], BF16, tag="sbf", name="sbf")
                        nc.vector.tensor_scalar_add(sb[:], psS[mo][:],
                                                    b2["s"][:, mo:mo + 1])
                        ex = epool.tile([P, TB], F32, tag="exps", name="exps")
                        nc.scalar.activation(ex[:], psS[mo][:], AF.Exp,
                                             bias=b2["s"][:, mo:mo + 1],
                                             scale=1.0)
                        tmp = opool.tile([P, TB], F32, tag="tmp", name="tmp",
                                         bufs=6)
                        nc.vector.tensor_mul(tmp[:], bt_t[mo][:], ex[:])
                        s_tiles.append(sb)
                        exp_tiles.append(tmp)
                else:
                    for mo in range(MO):
                        bo = opool.tile([P, TB], F32, tag="bout", name="bout")
                        nc.vector.scalar_tensor_tensor(
                            bo[:], psS[mo][:], b2["t"][:, mo:mo + 1],
                            exp_tiles[mo][:], op0=ALU.add, op1=ALU.add)
                        nc.sync.dma_start(bout_d[mo * P:(mo + 1) * P, ncol],
                                          bo[:])

    nc.compile()
    return nc


def _get_nc():
    if "nc" not in _CACHE:
        _CACHE["nc"] = _build_nc()
    return _CACHE["nc"]


def _prep_in_maps(z, W1s, b1s, W2s, b2s, W1t, b1t, W2t, b2t):
    z = np.asarray(z, dtype=np.float32)
    a = z[:, 0::2]
    AT = np.ascontiguousarray(a.T).astype(NPBF)       # [HALF, B]
    BT = np.ascontiguousarray(z[:, 1::2].T)           # [HALF, B] f32

    def prep_w(w):
        return np.ascontiguousarray(np.asarray(w, dtype=np.float32)).astype(NPBF)

    def prep_b1(b):
        return np.ascontiguousarray(np.asarray(b, np.float32).reshape(MH, P).T)

    def prep_b2(b):
        return np.ascontiguousarray(np.asarray(b, np.float32).reshape(MO, P).T)

    shared = {
        "w1s": prep_w(W1s), "w2s": prep_w(W2s),
        "w1t": prep_w(W1t), "w2t": prep_w(W2t),
        "b1s": prep_b1(b1s), "b2s": prep_b2(b2s),
        "b1t": prep_b1(b1t), "b2t": prep_b2(b2t),
    }
    in_maps = []
    for c in range(NCORES):
        sl = slice(c * BC, (c + 1) * BC)
        in_maps.append({
            "at": np.ascontiguousarray(AT[:, sl]),
            "bt": np.ascontiguousarray(BT[:, sl]),
            **shared,
        })
    return z, a, in_maps


def kernel(z, W1s, b1s, W2s, b2s, W1t, b1t, W2t, b2t):
    z, a, in_maps = _prep_in_maps(z, W1s, b1s, W2s, b2s, W1t, b1t, W2t, b2t)
    nc = _get_nc()
    res = None
    for attempt in range(3):
        try:
            res = bass_utils.run_bass_kernel_spmd(nc, in_maps,
                                                  core_ids=list(range(NCORES)))
            break
        except Exception:
            if attempt == 2:
                raise
            import time
            time.sleep(5)

    boutt = np.concatenate([r["boutt"] for r in res.results], axis=1)  # [HALF, B]
    logdet = np.concatenate([r["logdet"][0] for r in res.results])     # [B]

    z_out = np.empty_like(z)
    z_out[:, 0::2] = a
    z_out[:, 1::2] = boutt.T
    return z_out, logdet
